# revision 1
# baseline (speedup 1.0000x reference)
"""Trainium2 Bass kernel for DiffSelfAttention (B=1, T=2048, C=2048, 16 v-heads).

Sharding: tensor-parallel over heads across 8 NeuronCores. Core c owns
v-heads {2c, 2c+1} plus the matching q/k heads of both differential branches.
Each core computes its qkv slice, the attention for its 4 q/k heads, the
differential + per-head RMSNorm, and a partial projection
y_c = out_c @ w_proj[rows_c]. The host sums the 8 partials (unshard step).

Layout/strategy notes:
  - All matmuls run as float32r (full-rate fp32 on the PE at N>=256,
    ~2e-4 element rounding). DMA loads go directly into fp32r tiles;
    on-chip fp32r operands are produced by compute ops (engines round on
    write), which is what the BIR verifier requires.
  - q/k are produced directly transposed ([d, T]); v in natural layout
    ([T, d]); scores computed transposed ([tk, tq]) so probs@v needs no
    transposes anywhere.
  - Softmax divisions are eliminated: RMSNorm is invariant to any
    per-column positive scale, so instead of a1/r1 - lam*a2/r2 we feed it
    o' = a1*r2 - lam*a2*r1 (r = exp-sum broadcasts from a ones-matmul).
    The 1e-6 RMS eps is dropped: mean(o'^2) >> eps always for this data.
  - rsqrt for RMS is computed as exp(-0.5*log(m)) on the ACT engine
    (Reciprocal/Rsqrt activations are banned; Log+Exp share one ACT
    table set so there are no mid-kernel table switches).
  - Softmax column sums use two interleaved DVE accumulator chains so the
    serial dependency never gates the ACT exp stream.
"""

import math

import numpy as np

import concourse.bass as bass
import concourse.bacc as bacc
import concourse.mybir as mybir
import concourse.tile as tile

F32 = mybir.dt.float32
F32R = mybir.dt.float32r

T = 2048
C = 2048
N_HEAD = 16
H_DIM = 64
D2 = 2 * H_DIM  # 128 (v-head dim, also the RMS group size)
LAMBDA_INIT = 0.8 - 0.6 * math.exp(-0.3)
SCALE = 1.0 / math.sqrt(H_DIM)
P = 128
KSLABS = C // P  # 16 contraction slabs
TT = T // P  # 16 t-tiles
NCH = 512  # moving-operand chunk (max for 4-byte dtypes)
HQ = T // 2  # 1024-wide tq halves in the attention inner loop
N_CORES = 8

EXP = mybir.ActivationFunctionType.Exp
LOG = mybir.ActivationFunctionType.Ln
MULT = mybir.AluOpType.mult
ADD = mybir.AluOpType.add


def build(lam: float) -> bass.Bass:
    nc = bacc.Bacc("TRN2", target_bir_lowering=False, debug=False)

    xt_d = nc.dram_tensor("xt", [P, 4, KSLABS, NCH], F32R, kind="ExternalInput")
    wqk_d = nc.dram_tensor("wqk", [P, KSLABS, 4 * P], F32R, kind="ExternalInput")
    wv_d = nc.dram_tensor("wv", [P, KSLABS, 2 * D2], F32R, kind="ExternalInput")
    wp_d = nc.dram_tensor("wp", [P, 2, T], F32R, kind="ExternalInput")
    sv_d = nc.dram_tensor("sv", [P, 1], F32, kind="ExternalInput")
    y_d = nc.dram_tensor("y", [TT, P, T], F32, kind="ExternalOutput")

    with tile.TileContext(nc) as tc:
        with tc.tile_pool(name="persist", bufs=1) as persist:
            sv = persist.tile([P, 1], F32)
            ones_f = persist.tile([P, P], F32)
            ones = persist.tile([P, P], F32R)
            qk = persist.tile([P, 4, T], F32R)  # q1|q2|k1|k2, [d, T] layout
            vnat = persist.tile([P, TT, 2 * D2], F32R)  # v, [T, d] layout
            nc.sync.dma_start(out=sv, in_=sv_d[:])
            nc.vector.memset(ones_f, 1.0)
            nc.vector.tensor_copy(ones, ones_f)

            # ---------- phase 1: qkv projections ----------
            with tc.tile_pool(name="w1", bufs=1) as w1p, \
                 tc.tile_pool(name="xt", bufs=2) as xtp, \
                 tc.tile_pool(name="ps_qk", bufs=2, space="PSUM") as pqk, \
                 tc.tile_pool(name="ps_v", bufs=2, space="PSUM") as pvp:
                wqk = w1p.tile([P, KSLABS, 4 * P], F32R)
                wv = w1p.tile([P, KSLABS, 2 * D2], F32R)
                nc.sync.dma_start(out=wqk, in_=wqk_d[:])
                nc.sync.dma_start(out=wv, in_=wv_d[:])
                for n in range(T // NCH):  # 512-wide t chunks
                    xt = xtp.tile([P, KSLABS, NCH], F32R)
                    nc.sync.dma_start(out=xt, in_=xt_d[:, n, :, :])
                    for m in range(4):  # q1, q2, k1, k2
                        ps = pqk.tile([P, NCH], F32)
                        for k in range(KSLABS):
                            nc.tensor.matmul(
                                ps,
                                wqk[:, k, m * P:(m + 1) * P],
                                xt[:, k, :],
                                start=(k == 0),
                                stop=(k == KSLABS - 1),
                            )
                        nc.vector.tensor_copy(qk[:, m, n * NCH:(n + 1) * NCH], ps)
                    for t2 in range(NCH // P):  # t-tiles in this chunk
                        ps = pvp.tile([P, 2 * D2], F32)
                        for k in range(KSLABS):
                            nc.tensor.matmul(
                                ps,
                                xt[:, k, t2 * P:(t2 + 1) * P],
                                wv[:, k, :],
                                start=(k == 0),
                                stop=(k == KSLABS - 1),
                            )
                        nc.vector.tensor_copy(vnat[:, n * (NCH // P) + t2, :], ps)

            # ---------- phases 2+3 ----------
            with tc.tile_pool(name="wp", bufs=1) as wpp:
                wp = wpp.tile([P, 2, T], F32R)
                on = wpp.tile([P, 2, T], F32R)  # normed diff out, [d, T] per vh
                nc.sync.dma_start(out=wp, in_=wp_d[:])

                # ---------- phase 2: attention ----------
                # Both v-head streams (array rows 0-63 / 64-127) are packed
                # into shared [P, 2, NCH] tiles: one ACT exp covers both, and
                # the PE gets 6 matmuls per tk-slab (scores x2, pv x2,
                # colsum x2) so it never idles long enough for the HAM
                # clock-gate to re-throttle it to 1.2 GHz.
                with tc.tile_pool(name="ps_s", bufs=2, space="PSUM") as psp, \
                     tc.tile_pool(name="ps_a", bufs=1, space="PSUM") as pap, \
                     tc.tile_pool(name="ps_r", bufs=1, space="PSUM") as rp, \
                     tc.tile_pool(name="exp", bufs=4) as ep, \
                     tc.tile_pool(name="keep", bufs=1) as kp:
                    opk = kp.tile([P, 2, T], F32)  # scaled diff o', per vh
                    a1u = {}
                    r1l = {}
                    for br in range(2):
                        for q4 in range(4):  # 512-wide tq quarters
                            c0 = q4 * NCH
                            pa = pap.tile([P, 2, NCH], F32, tag="pa")
                            r = rp.tile([P, 2, NCH], F32, tag="r")
                            for k in range(TT):  # tk slabs
                                ps = psp.tile([P, 2, NCH], F32, tag="s")
                                et = ep.tile([P, 2, NCH], F32R, tag="er")
                                for vh in range(2):
                                    rows = slice(vh * H_DIM, (vh + 1) * H_DIM)
                                    nc.tensor.matmul(
                                        ps[:, vh, :],
                                        qk[rows, 2 + br, k * P:(k + 1) * P],
                                        qk[rows, br, c0:c0 + NCH],
                                        start=True,
                                        stop=True,
                                    )
                                nc.scalar.activation(et, ps, EXP, scale=SCALE)
                                for vh in range(2):
                                    nc.tensor.matmul(
                                        pa[:, vh, :],
                                        vnat[:, k, vh * D2:(vh + 1) * D2],
                                        et[:, vh, :],
                                        start=(k == 0),
                                        stop=(k == TT - 1),
                                    )
                                    nc.tensor.matmul(
                                        r[:, vh, :],
                                        ones,
                                        et[:, vh, :],
                                        start=(k == 0),
                                        stop=(k == TT - 1),
                                    )
                            if br == 0:
                                # keep unnormalized a1 and -lam*r1 for branch 2
                                a1u[q4] = kp.tile([P, 2, NCH], F32, tag=f"a1u{q4}", name=f"a1u{q4}")
                                nc.vector.tensor_copy(a1u[q4], pa)
                                r1l[q4] = kp.tile([P, 2, NCH], F32, tag=f"r1l{q4}", name=f"r1l{q4}")
                                nc.vector.tensor_scalar_mul(r1l[q4], r, -lam)
                            else:
                                # o' = a1*r2 - lam*a2*r1  (a per-column positive
                                # rescale of o; RMSNorm cancels it)
                                m1 = ep.tile([P, 2, NCH], F32, tag="m1")
                                nc.vector.tensor_mul(m1, a1u[q4], r)
                                m2 = ep.tile([P, 2, NCH], F32, tag="m2")
                                nc.vector.tensor_mul(m2, pa, r1l[q4])
                                nc.vector.tensor_add(opk[:, :, c0:c0 + NCH], m1, m2)
                    # RMS: rsqrt(mean o'^2) = exp(-0.5*ln(mean)). All Ln ops
                    # emitted before all Exp ops -> at most 2 ACT table loads.
                    psms = []
                    for vh in range(2):
                        for hf in range(2):
                            q0 = hf * HQ
                            sq = ep.tile([P, HQ], F32R, tag="er")
                            nc.vector.tensor_mul(sq, opk[:, vh, q0:q0 + HQ], opk[:, vh, q0:q0 + HQ])
                            psm = psp.tile([P, HQ], F32, tag="s")
                            for c2 in range(2):
                                nc.tensor.matmul(
                                    psm[:, c2 * NCH:(c2 + 1) * NCH],
                                    ones,
                                    sq[:, c2 * NCH:(c2 + 1) * NCH],
                                    start=True,
                                    stop=True,
                                )
                            ln = kp.tile([P, HQ], F32, tag=f"a1u{2 * vh + hf}", name=f"ln{vh}{hf}")
                            nc.scalar.activation(ln, psm, LOG, scale=1.0 / D2)
                            psms.append(ln)
                    for vh in range(2):
                        for hf in range(2):
                            q0 = hf * HQ
                            rsq = ep.tile([P, HQ], F32, tag="m1")
                            nc.scalar.activation(rsq, psms[2 * vh + hf], EXP, scale=-0.5)
                            nc.vector.scalar_tensor_tensor(
                                on[:, vh, q0:q0 + HQ],
                                opk[:, vh, q0:q0 + HQ],
                                sv, rsq, op0=MULT, op1=MULT,
                            )

                # ---------- phase 3: output projection (partial sum) ----------
                with tc.tile_pool(name="ps_y", bufs=4, space="PSUM") as pyp, \
                     tc.tile_pool(name="ysb", bufs=3) as yp:
                    for tt_i in range(TT):
                        ysb = yp.tile([P, T], F32)
                        for nch in range(T // NCH):
                            py = pyp.tile([P, NCH], F32)
                            for vh in range(2):
                                nc.tensor.matmul(
                                    py,
                                    on[:, vh, tt_i * P:(tt_i + 1) * P],
                                    wp[:, vh, nch * NCH:(nch + 1) * NCH],
                                    start=(vh == 0),
                                    stop=(vh == 1),
                                )
                            nc.vector.tensor_copy(ysb[:, nch * NCH:(nch + 1) * NCH], py)
                        nc.sync.dma_start(out=y_d[tt_i], in_=ysb)
    nc.finalize()
    return nc


def _core_inputs(x, w_qkv, w_proj, rms_scale):
    """Host-side shard prep: per-core weight slices + replicated x^T."""
    xt = np.ascontiguousarray(x.reshape(T, C).T)  # [C, T]
    xtr = np.ascontiguousarray(
        xt.reshape(KSLABS, P, T // NCH, NCH).transpose(1, 2, 0, 3)
    )
    sv = np.ascontiguousarray(
        (rms_scale.astype(np.float32) * np.float32(1.0 - LAMBDA_INIT)).reshape(P, 1)
    )
    maps = []
    for c in range(N_CORES):
        cols = [
            w_qkv[:, 0 * 1024 + c * P:0 * 1024 + (c + 1) * P],  # q1 heads 2c,2c+1
            w_qkv[:, 1 * 1024 + c * P:1 * 1024 + (c + 1) * P],  # q2
            w_qkv[:, 2 * 1024 + c * P:2 * 1024 + (c + 1) * P],  # k1
            w_qkv[:, 3 * 1024 + c * P:3 * 1024 + (c + 1) * P],  # k2
        ]
        wqk = np.concatenate(cols, axis=1)  # [C, 512]
        wqk = np.ascontiguousarray(wqk.reshape(KSLABS, P, 4 * P).transpose(1, 0, 2))
        wv = w_qkv[:, 2 * C + c * 2 * D2:2 * C + (c + 1) * 2 * D2]  # [C, 256]
        wv = np.ascontiguousarray(wv.reshape(KSLABS, P, 2 * D2).transpose(1, 0, 2))
        wp = w_proj[c * 2 * D2:(c + 1) * 2 * D2, :]  # [256, T]
        wp = np.ascontiguousarray(wp.reshape(2, P, T).transpose(1, 0, 2))
        maps.append({"xt": xtr, "wqk": wqk, "wv": wv, "wp": wp, "sv": sv})
    return maps


def kernel(x, w_qkv, w_proj, lambda_q1, lambda_k1, lambda_q2, lambda_k2, rms_scale):
    from concourse.bass_utils import run_bass_kernel_spmd

    x = np.asarray(x, dtype=np.float32)
    w_qkv = np.asarray(w_qkv, dtype=np.float32)
    w_proj = np.asarray(w_proj, dtype=np.float32)
    rms_scale = np.asarray(rms_scale, dtype=np.float32)
    lam1 = np.exp(np.sum(np.asarray(lambda_q1) * np.asarray(lambda_k1), dtype=np.float32))
    lam2 = np.exp(np.sum(np.asarray(lambda_q2) * np.asarray(lambda_k2), dtype=np.float32))
    lam = float(lam1 - lam2 + LAMBDA_INIT)

    nc = build(lam)
    in_maps = _core_inputs(x, w_qkv, w_proj, rms_scale)
    res = run_bass_kernel_spmd(nc, in_maps, core_ids=list(range(N_CORES)))
    y = np.zeros((TT, P, T), np.float32)
    for rmap in res.results:
        y += rmap["y"]
    return y.reshape(1, T, C)



# revision 8
# speedup vs baseline: 1.2452x; 1.2452x over previous
"""Trainium2 Bass kernel for DiffSelfAttention (B=1, T=2048, C=2048, 16 v-heads).

Sharding: tensor-parallel over heads across 8 NeuronCores. Core c owns
v-heads {2c, 2c+1} plus the matching q/k heads of both differential branches.
Each core computes its qkv slice, the attention for its 4 q/k heads, the
differential + per-head RMSNorm, and a partial projection
y_c = out_c @ w_proj[rows_c]. The host sums the 8 partials (unshard step).

Layout/strategy notes:
  - All matmuls run as float32r (full-rate fp32 on the PE at N>=256,
    ~2e-4 element rounding). DMA loads go directly into fp32r tiles;
    on-chip fp32r operands are produced by compute ops (engines round on
    write), which is what the BIR verifier requires.
  - q/k are produced directly transposed ([d, T]); v in natural layout
    ([T, d]); scores computed transposed ([tk, tq]) so probs@v needs no
    transposes anywhere.
  - Softmax divisions are eliminated: RMSNorm is invariant to any
    per-column positive scale, so instead of a1/r1 - lam*a2/r2 we feed it
    o' = a1*r2 - lam*a2*r1 (r = exp-sum broadcasts from a ones-matmul).
    The 1e-6 RMS eps is dropped: mean(o'^2) >> eps always for this data.
  - rsqrt for RMS is computed as exp(-0.5*log(m)) on the ACT engine
    (Reciprocal/Rsqrt activations are banned; Log+Exp share one ACT
    table set so there are no mid-kernel table switches).
  - Softmax column sums use two interleaved DVE accumulator chains so the
    serial dependency never gates the ACT exp stream.
"""

import math

import numpy as np

import concourse.bass as bass
import concourse.bacc as bacc
import concourse.mybir as mybir
import concourse.tile as tile

F32 = mybir.dt.float32
F32R = mybir.dt.float32r

T = 2048
C = 2048
N_HEAD = 16
H_DIM = 64
D2 = 2 * H_DIM  # 128 (v-head dim, also the RMS group size)
LAMBDA_INIT = 0.8 - 0.6 * math.exp(-0.3)
SCALE = 1.0 / math.sqrt(H_DIM)
P = 128
KSLABS = C // P  # 16 contraction slabs
TT = T // P  # 16 t-tiles
NCH = 512  # moving-operand chunk (max for 4-byte dtypes)
HQ = T // 2  # 1024-wide tq halves in the attention inner loop
N_CORES = 8

EXP = mybir.ActivationFunctionType.Exp
LOG = mybir.ActivationFunctionType.Ln
MULT = mybir.AluOpType.mult
ADD = mybir.AluOpType.add
BF16 = mybir.dt.bfloat16
DMAG = 2  # k-slab granularity of the startup weight/x DMA stream


def build(lam: float) -> bass.Bass:
    nc = bacc.Bacc("TRN2", target_bir_lowering=False, debug=False)

    xt_d = nc.dram_tensor("xt", [P, 4, KSLABS, NCH], F32R, kind="ExternalInput")
    wqk_d = nc.dram_tensor("wqk", [P, KSLABS, 4 * P], F32R, kind="ExternalInput")
    wv_d = nc.dram_tensor("wv", [P, KSLABS, 2 * D2], F32R, kind="ExternalInput")
    wp_d = nc.dram_tensor("wp", [P, 2, T], F32R, kind="ExternalInput")
    sv_d = nc.dram_tensor("sv", [P, 1], F32, kind="ExternalInput")
    y_d = nc.dram_tensor("y", [TT, P, T], BF16, kind="ExternalOutput")

    with tile.TileContext(nc) as tc:
        with tc.tile_pool(name="persist", bufs=1) as persist:
            sv = persist.tile([P, 1], F32)
            ones_f = persist.tile([P, P], F32)
            ones = persist.tile([P, P], F32R)
            qk = persist.tile([P, 4, T], F32R)  # q1|q2|k1|k2, [d, T] layout
            vnat = persist.tile([P, TT, 2 * D2], F32R)  # v, [T, d] layout
            nc.sync.dma_start(out=sv, in_=sv_d[:])
            nc.vector.memset(ones_f, 1.0)
            nc.vector.tensor_copy(ones, ones_f)

            # ---------- phase 1: qkv projections ----------
            with tc.tile_pool(name="w1", bufs=1) as w1p, \
                 tc.tile_pool(name="xt", bufs=2) as xtp, \
                 tc.tile_pool(name="ps_qk", bufs=2, space="PSUM") as pqk, \
                 tc.tile_pool(name="ps_v", bufs=2, space="PSUM") as pvp:
                wqk = w1p.tile([P, KSLABS, 4 * P], F32R)
                wv = w1p.tile([P, KSLABS, 2 * D2], F32R)
                for n in range(T // NCH):  # 512-wide t chunks
                    xt = xtp.tile([P, KSLABS, NCH], F32R)
                    if n == 0:
                        # Stream wqk + x chunk 0 in small interleaved k-slab
                        # groups so the first matmul starts ~3us in (one 4MB
                        # DMA each would stall the PE ~40us at kernel start).
                        for g in range(KSLABS // DMAG):
                            sl = slice(g * DMAG, (g + 1) * DMAG)
                            nc.sync.dma_start(out=wqk[:, sl, :], in_=wqk_d[:, sl, :])
                            nc.sync.dma_start(out=xt[:, sl, :], in_=xt_d[:, 0, sl, :])
                        nc.sync.dma_start(out=wv, in_=wv_d[:])
                    else:
                        nc.sync.dma_start(out=xt, in_=xt_d[:, n, :, :])
                    for m in range(4):  # q1, q2, k1, k2
                        ps = pqk.tile([P, NCH], F32)
                        for k in range(KSLABS):
                            nc.tensor.matmul(
                                ps,
                                wqk[:, k, m * P:(m + 1) * P],
                                xt[:, k, :],
                                start=(k == 0),
                                stop=(k == KSLABS - 1),
                            )
                        nc.vector.tensor_copy(qk[:, m, n * NCH:(n + 1) * NCH], ps)
                    for t2 in range(NCH // P):  # t-tiles in this chunk
                        ps = pvp.tile([P, 2 * D2], F32)
                        for k in range(KSLABS):
                            nc.tensor.matmul(
                                ps,
                                xt[:, k, t2 * P:(t2 + 1) * P],
                                wv[:, k, :],
                                start=(k == 0),
                                stop=(k == KSLABS - 1),
                            )
                        nc.vector.tensor_copy(vnat[:, n * (NCH // P) + t2, :], ps)

            # ---------- phases 2+3 ----------
            with tc.tile_pool(name="wp", bufs=1) as wpp:
                wp = wpp.tile([P, 2, T], F32R)
                on = wpp.tile([P, 2, T], F32R)  # normed diff out, [d, T] per vh
                nc.sync.dma_start(out=wp, in_=wp_d[:])

                # ---------- phase 2: attention ----------
                # Both v-head streams (array rows 0-63 / 64-127) are packed
                # into shared [P, 2, NCH] tiles: one ACT exp covers both, and
                # the PE gets 6 matmuls per tk-slab (scores x2, pv x2,
                # colsum x2) so it never idles long enough for the HAM
                # clock-gate to re-throttle it to 1.2 GHz.
                with tc.tile_pool(name="ps_s", bufs=2, space="PSUM") as psp, \
                     tc.tile_pool(name="ps_a", bufs=1, space="PSUM") as pap, \
                     tc.tile_pool(name="ps_r", bufs=1, space="PSUM") as rp, \
                     tc.tile_pool(name="exp", bufs=4) as ep, \
                     tc.tile_pool(name="keep", bufs=1) as kp:
                    opk = kp.tile([P, 2, T], F32)  # scaled diff o', per vh
                    a1u = {}
                    r1l = {}
                    for br in range(2):
                        for q4 in range(4):  # 512-wide tq quarters
                            c0 = q4 * NCH
                            pa = pap.tile([P, 2, NCH], F32, tag="pa")
                            r = rp.tile([P, 2, NCH], F32, tag="r")

                            def scores(k):
                                ps = psp.tile([P, 2, NCH], F32, tag="s")
                                for vh in range(2):
                                    rows = slice(vh * H_DIM, (vh + 1) * H_DIM)
                                    nc.tensor.matmul(
                                        ps[:, vh, :],
                                        qk[rows, 2 + br, k * P:(k + 1) * P],
                                        qk[rows, br, c0:c0 + NCH],
                                        start=True,
                                        stop=True,
                                    )
                                return ps

                            # Software pipeline: emit scores(k+1) on the PE
                            # BEFORE pv(k)/r(k), so the PE streams scores while
                            # the ACT engine exps slab k. Without the lookahead
                            # the chain s(k)->exp(k)->pv(k)->s(k+1) serializes
                            # (~2.4us/slab instead of ~1.4us).
                            ps_cur = scores(0)
                            for k in range(TT):  # tk slabs
                                ps_nxt = scores(k + 1) if k + 1 < TT else None
                                et = ep.tile([P, 2, NCH], F32R, tag="er")
                                nc.scalar.activation(et, ps_cur, EXP, scale=SCALE)
                                for vh in range(2):
                                    nc.tensor.matmul(
                                        pa[:, vh, :],
                                        vnat[:, k, vh * D2:(vh + 1) * D2],
                                        et[:, vh, :],
                                        start=(k == 0),
                                        stop=(k == TT - 1),
                                    )
                                    nc.tensor.matmul(
                                        r[:, vh, :],
                                        ones,
                                        et[:, vh, :],
                                        start=(k == 0),
                                        stop=(k == TT - 1),
                                    )
                                ps_cur = ps_nxt
                            if br == 0:
                                # keep unnormalized a1 and -lam*r1 for branch 2
                                # (a1u on DVE frees pa; r1l on ACT frees r — the
                                # two evacuations run on different engines so the
                                # next group's accumulations start sooner)
                                a1u[q4] = kp.tile([P, 2, NCH], F32, tag=f"a1u{q4}", name=f"a1u{q4}")
                                nc.vector.tensor_copy(a1u[q4], pa)
                                r1l[q4] = kp.tile([P, 2, NCH], F32, tag=f"r1l{q4}", name=f"r1l{q4}")
                                nc.scalar.mul(r1l[q4], r, -lam)
                            else:
                                # o' = a1*r2 - lam*a2*r1  (a per-column positive
                                # rescale of o; RMSNorm cancels it). m2 first:
                                # it reads pa, which gates the next group's pv.
                                m2 = ep.tile([P, 2, NCH], F32, tag="m2")
                                nc.vector.tensor_mul(m2, pa, r1l[q4])
                                m1 = ep.tile([P, 2, NCH], F32, tag="m1")
                                nc.vector.tensor_mul(m1, a1u[q4], r)
                                nc.vector.tensor_add(opk[:, :, c0:c0 + NCH], m1, m2)
                    # RMS: rsqrt(mean o'^2) = exp(-0.5*ln(mean)). All Ln ops
                    # emitted before all Exp ops -> at most 2 ACT table loads.
                    psms = []
                    for vh in range(2):
                        for hf in range(2):
                            q0 = hf * HQ
                            sq = ep.tile([P, HQ], F32R, tag="er")
                            nc.vector.tensor_mul(sq, opk[:, vh, q0:q0 + HQ], opk[:, vh, q0:q0 + HQ])
                            psm = psp.tile([P, HQ], F32, tag="s")
                            for c2 in range(2):
                                nc.tensor.matmul(
                                    psm[:, c2 * NCH:(c2 + 1) * NCH],
                                    ones,
                                    sq[:, c2 * NCH:(c2 + 1) * NCH],
                                    start=True,
                                    stop=True,
                                )
                            ln = kp.tile([P, HQ], F32, tag=f"a1u{2 * vh + hf}", name=f"ln{vh}{hf}")
                            nc.scalar.activation(ln, psm, LOG, scale=1.0 / D2)
                            psms.append(ln)
                    for vh in range(2):
                        for hf in range(2):
                            q0 = hf * HQ
                            rsq = ep.tile([P, HQ], F32, tag="m1")
                            nc.scalar.activation(rsq, psms[2 * vh + hf], EXP, scale=-0.5)
                            nc.vector.scalar_tensor_tensor(
                                on[:, vh, q0:q0 + HQ],
                                opk[:, vh, q0:q0 + HQ],
                                sv, rsq, op0=MULT, op1=MULT,
                            )

                # ---------- phase 3: output projection (partial sum) ----------
                with tc.tile_pool(name="ps_y", bufs=4, space="PSUM") as pyp, \
                     tc.tile_pool(name="ysb", bufs=3) as yp:
                    for tt_i in range(TT):
                        ysb = yp.tile([P, T], BF16)
                        for nch in range(T // NCH):
                            py = pyp.tile([P, NCH], F32)
                            for vh in range(2):
                                nc.tensor.matmul(
                                    py,
                                    on[:, vh, tt_i * P:(tt_i + 1) * P],
                                    wp[:, vh, nch * NCH:(nch + 1) * NCH],
                                    start=(vh == 0),
                                    stop=(vh == 1),
                                )
                            nc.vector.tensor_copy(ysb[:, nch * NCH:(nch + 1) * NCH], py)
                        nc.sync.dma_start(out=y_d[tt_i], in_=ysb)
    nc.finalize()
    return nc


def _core_inputs(x, w_qkv, w_proj, rms_scale):
    """Host-side shard prep: per-core weight slices + replicated x^T."""
    xt = np.ascontiguousarray(x.reshape(T, C).T)  # [C, T]
    xtr = np.ascontiguousarray(
        xt.reshape(KSLABS, P, T // NCH, NCH).transpose(1, 2, 0, 3)
    )
    sv = np.ascontiguousarray(
        (rms_scale.astype(np.float32) * np.float32(1.0 - LAMBDA_INIT)).reshape(P, 1)
    )
    maps = []
    for c in range(N_CORES):
        cols = [
            w_qkv[:, 0 * 1024 + c * P:0 * 1024 + (c + 1) * P],  # q1 heads 2c,2c+1
            w_qkv[:, 1 * 1024 + c * P:1 * 1024 + (c + 1) * P],  # q2
            w_qkv[:, 2 * 1024 + c * P:2 * 1024 + (c + 1) * P],  # k1
            w_qkv[:, 3 * 1024 + c * P:3 * 1024 + (c + 1) * P],  # k2
        ]
        wqk = np.concatenate(cols, axis=1)  # [C, 512]
        wqk = np.ascontiguousarray(wqk.reshape(KSLABS, P, 4 * P).transpose(1, 0, 2))
        wv = w_qkv[:, 2 * C + c * 2 * D2:2 * C + (c + 1) * 2 * D2]  # [C, 256]
        wv = np.ascontiguousarray(wv.reshape(KSLABS, P, 2 * D2).transpose(1, 0, 2))
        wp = w_proj[c * 2 * D2:(c + 1) * 2 * D2, :]  # [256, T]
        wp = np.ascontiguousarray(wp.reshape(2, P, T).transpose(1, 0, 2))
        maps.append({"xt": xtr, "wqk": wqk, "wv": wv, "wp": wp, "sv": sv})
    return maps


def kernel(x, w_qkv, w_proj, lambda_q1, lambda_k1, lambda_q2, lambda_k2, rms_scale):
    from concourse.bass_utils import run_bass_kernel_spmd

    x = np.asarray(x, dtype=np.float32)
    w_qkv = np.asarray(w_qkv, dtype=np.float32)
    w_proj = np.asarray(w_proj, dtype=np.float32)
    rms_scale = np.asarray(rms_scale, dtype=np.float32)
    lam1 = np.exp(np.sum(np.asarray(lambda_q1) * np.asarray(lambda_k1), dtype=np.float32))
    lam2 = np.exp(np.sum(np.asarray(lambda_q2) * np.asarray(lambda_k2), dtype=np.float32))
    lam = float(lam1 - lam2 + LAMBDA_INIT)

    nc = build(lam)
    in_maps = _core_inputs(x, w_qkv, w_proj, rms_scale)
    res = run_bass_kernel_spmd(nc, in_maps, core_ids=list(range(N_CORES)))
    y = np.zeros((TT, P, T), np.float32)
    for rmap in res.results:
        y += np.asarray(rmap["y"], dtype=np.float32)
    return y.reshape(1, T, C)



# revision 12
# speedup vs baseline: 1.4565x; 1.1697x over previous
"""Trainium2 Bass kernel for DiffSelfAttention (B=1, T=2048, C=2048, 16 v-heads).

Sharding: tensor-parallel over heads across 8 NeuronCores. Core c owns
v-heads {2c, 2c+1} plus the matching q/k heads of both differential branches.
Each core computes its qkv slice, the attention for its 4 q/k heads, the
differential + per-head RMSNorm, and a partial projection
y_c = out_c @ w_proj[rows_c]. The host sums the 8 partials (unshard step).

Layout/strategy notes (v3):
  - Everything that feeds the PE is bf16 (x, w_qkv slices, q/k/v, exp(scores),
    w_proj, normed out). PSUM accumulation stays fp32, so only input rounding
    (~0.1% rms) enters; measured end-to-end rel err ~2e-3 vs the 2e-2 budget.
    bf16 also enables Fast Weight Load on LDWEIGHTS and 2x DVE modes.
  - q/k produced transposed ([d, T]); v natural ([T, d]); scores computed
    transposed ([tk, tq]) so probs@v needs no transposes anywhere.
  - Softmax divisions eliminated: RMSNorm is invariant to per-column positive
    scale, so o' = a1*r2 - lam*a2*r1 replaces a1/r1 - lam*a2/r2.
  - The attention inner loop is software-pipelined: scores(k+1) is emitted on
    the PE before pv(k), so the PE streams scores while ACT exps slab k.
  - exp-sum r is accumulated on the DVE (bf16 adds over the et ring) with one
    final ones-matmul for the cross-partition reduction; this removes 131k PE
    cycles and frees 2 PSUM banks vs accumulating r on the PE.
  - RMSNorm (exp(-0.5*ln(mean)) on ACT) and the output projection run
    per-512-column group; the projection matmuls/copies/DMAs of group g are
    emitted one step per slab-beat inside group g+1, hiding them in the
    PE/ACT slack and spreading the output DMA across the whole kernel.
  - ~36 ones@ones warm-up matmuls at kernel start keep the PE busy while the
    first weight DMAs land, so the HAM clock-gate releases (1.2->2.4 GHz)
    right as real work begins. A dummy exp prepays the ACT table load.
"""

import math

import numpy as np
import ml_dtypes

import concourse.bass as bass
import concourse.bacc as bacc
import concourse.mybir as mybir
import concourse.tile as tile

F32 = mybir.dt.float32
F32R = mybir.dt.float32r
BF16 = mybir.dt.bfloat16

T = 2048
C = 2048
N_HEAD = 16
H_DIM = 64
D2 = 2 * H_DIM  # 128 (v-head dim, also the RMS group size)
LAMBDA_INIT = 0.8 - 0.6 * math.exp(-0.3)
SCALE = 1.0 / math.sqrt(H_DIM)
P = 128
KSLABS = C // P  # 16 contraction slabs
TT = T // P  # 16 t-tiles
NCH = 512  # tq chunk = PSUM bank width in fp32
N_CORES = 8
DMAG = 2  # k-slab granularity of the startup weight/x DMA stream
NWARM = 36  # HAM warm-up matmuls

EXP = mybir.ActivationFunctionType.Exp
LOG = mybir.ActivationFunctionType.Ln
MULT = mybir.AluOpType.mult


def build(lam: float) -> bass.Bass:
    nc = bacc.Bacc("TRN2", target_bir_lowering=False, debug=False)

    xt_d = nc.dram_tensor("xt", [P, 4, KSLABS, NCH], BF16, kind="ExternalInput")
    wqk_d = nc.dram_tensor("wqk", [P, KSLABS, 4 * P], BF16, kind="ExternalInput")
    wv_d = nc.dram_tensor("wv", [P, KSLABS, 2 * D2], BF16, kind="ExternalInput")
    wp_d = nc.dram_tensor("wp", [P, 2, T], BF16, kind="ExternalInput")
    sv_d = nc.dram_tensor("sv", [P, 1], F32, kind="ExternalInput")
    y_d = nc.dram_tensor("y", [TT, P, T], BF16, kind="ExternalOutput")

    with tile.TileContext(nc) as tc:
        with tc.tile_pool(name="persist", bufs=1) as persist:
            sv = persist.tile([P, 1], F32)
            ones_f = persist.tile([P, P], F32)
            ones = persist.tile([P, P], BF16)
            actw = persist.tile([P, 1], F32)
            qk = persist.tile([P, 4, T], BF16)  # q1|q2|k1|k2, [d, T] layout
            vnat = persist.tile([P, TT, 2 * D2], BF16)  # v, [T, d] layout
            nc.sync.dma_start(out=sv, in_=sv_d[:])
            nc.vector.memset(ones_f, 1.0)
            nc.vector.tensor_copy(ones, ones_f)
            # prepay the exp table load while the PE warms up
            nc.scalar.activation(actw, ones_f[:, 0:1], EXP, scale=1.0)

            # ---------- phase 1: qkv projections ----------
            with tc.tile_pool(name="w1", bufs=1) as w1p, \
                 tc.tile_pool(name="xt", bufs=2) as xtp, \
                 tc.tile_pool(name="ps_qk", bufs=2, space="PSUM") as pqk, \
                 tc.tile_pool(name="ps_v", bufs=2, space="PSUM") as pvp:
                warm = pqk.tile([P, P], F32, tag="warm", bufs=1)
                for _ in range(NWARM):
                    nc.tensor.matmul(warm, ones, ones, start=True, stop=True)
                wqk = w1p.tile([P, KSLABS, 4 * P], BF16)
                wv = w1p.tile([P, KSLABS, 2 * D2], BF16)
                for n in range(T // NCH):  # 512-wide t chunks
                    xt = xtp.tile([P, KSLABS, NCH], BF16)
                    if n == 0:
                        # Stream wqk + x chunk 0 in small interleaved k-slab
                        # groups so the first matmul starts a few us in (one
                        # big DMA each would stall the PE ~40us at start).
                        for g in range(KSLABS // DMAG):
                            sl = slice(g * DMAG, (g + 1) * DMAG)
                            nc.sync.dma_start(out=wqk[:, sl, :], in_=wqk_d[:, sl, :])
                            nc.sync.dma_start(out=xt[:, sl, :], in_=xt_d[:, 0, sl, :])
                        nc.sync.dma_start(out=wv, in_=wv_d[:])
                    else:
                        nc.sync.dma_start(out=xt, in_=xt_d[:, n, :, :])
                    for m in range(4):  # q1, q2, k1, k2
                        ps = pqk.tile([P, NCH], F32)
                        for k in range(KSLABS):
                            nc.tensor.matmul(
                                ps,
                                wqk[:, k, m * P:(m + 1) * P],
                                xt[:, k, :],
                                start=(k == 0),
                                stop=(k == KSLABS - 1),
                            )
                        nc.vector.tensor_copy(qk[:, m, n * NCH:(n + 1) * NCH], ps)
                    for t2 in range(NCH // P):  # t-tiles in this chunk
                        ps = pvp.tile([P, 2 * D2], F32)
                        for k in range(KSLABS):
                            nc.tensor.matmul(
                                ps,
                                xt[:, k, t2 * P:(t2 + 1) * P],
                                wv[:, k, :],
                                start=(k == 0),
                                stop=(k == KSLABS - 1),
                            )
                        nc.vector.tensor_copy(vnat[:, n * (NCH // P) + t2, :], ps)

            # ---------- phases 2+3, fused per 512-column group ----------
            with tc.tile_pool(name="wp", bufs=1) as wpp, \
                 tc.tile_pool(name="ps_s", bufs=2, space="PSUM") as psp, \
                 tc.tile_pool(name="ps_a", bufs=1, space="PSUM") as pap, \
                 tc.tile_pool(name="ps_y", bufs=2, space="PSUM") as pyp, \
                 tc.tile_pool(name="exp", bufs=2) as ep, \
                 tc.tile_pool(name="keep", bufs=1) as kp, \
                 tc.tile_pool(name="ysb", bufs=3) as yp:
                wp = wpp.tile([P, 2, T], BF16)
                nc.sync.dma_start(out=wp, in_=wp_d[:])

                # Deferred projection steps: emitted one per slab-beat so the
                # proj matmuls ride in the attention loop's dependency slack
                # instead of serializing between groups.
                pending = []

                def emit_proj(q4):
                    on_t = on[q4]
                    for t2 in range(NCH // P):
                        ysb = yp.tile([P, T], BF16, tag="ysb", name=f"ysb{q4}{t2}")
                        for nchk in range(T // NCH):
                            def step(t2=t2, nchk=nchk, ysb=ysb, on_t=on_t, q4=q4):
                                py = pyp.tile([P, NCH], F32, name="py")
                                for vh in range(2):
                                    nc.tensor.matmul(
                                        py,
                                        on_t[:, vh, t2 * P:(t2 + 1) * P],
                                        wp[:, vh, nchk * NCH:(nchk + 1) * NCH],
                                        start=(vh == 0),
                                        stop=(vh == 1),
                                    )
                                nc.vector.tensor_copy(
                                    ysb[:, nchk * NCH:(nchk + 1) * NCH], py)
                                if nchk == T // NCH - 1:
                                    nc.sync.dma_start(
                                        out=y_d[q4 * (NCH // P) + t2], in_=ysb)
                            pending.append(step)

                on = {}
                for q4 in range(4):
                    c0 = q4 * NCH
                    a1u = None
                    r1l = None
                    opk = None
                    for br in range(2):
                        pa = pap.tile([P, 2, NCH], F32, tag="pa", name="pa")
                        racc = kp.tile([P, 2, NCH], BF16, tag=f"racc{br}",
                                       name=f"racc{br}")

                        def scores(k):
                            ps = psp.tile([P, 2, NCH], F32, tag="s", name="ps")
                            for vh in range(2):
                                rows = slice(vh * H_DIM, (vh + 1) * H_DIM)
                                nc.tensor.matmul(
                                    ps[:, vh, :],
                                    qk[rows, 2 + br, k * P:(k + 1) * P],
                                    qk[rows, br, c0:c0 + NCH],
                                    start=True,
                                    stop=True,
                                )
                            return ps

                        ps_cur = scores(0)
                        for k in range(TT):  # tk slabs
                            ps_nxt = scores(k + 1) if k + 1 < TT else None
                            et = ep.tile([P, 2, NCH], BF16, tag="er", bufs=12,
                                         name="et")
                            nc.scalar.activation(et, ps_cur, EXP, scale=SCALE)
                            for vh in range(2):
                                nc.tensor.matmul(
                                    pa[:, vh, :],
                                    vnat[:, k, vh * D2:(vh + 1) * D2],
                                    et[:, vh, :],
                                    start=(k == 0),
                                    stop=(k == TT - 1),
                                )
                            if k == 0:
                                nc.vector.tensor_copy(racc, et)
                            else:
                                nc.vector.tensor_add(racc, racc, et)
                            if pending:
                                pending.pop(0)()
                            ps_cur = ps_nxt
                        # cross-partition reduce of the exp-sums (broadcasts
                        # the column sums to all 128 partitions)
                        rps = psp.tile([P, 2, NCH], F32, tag="s", name="rps")
                        for vh in range(2):
                            nc.tensor.matmul(rps[:, vh, :], ones, racc[:, vh, :],
                                             start=True, stop=True)
                        if br == 0:
                            # keep unnormalized a1 and -lam*r1 for branch 2
                            # (a1u stays f32: tensor_mul encodes one source
                            # dtype, so mixing bf16/f32 inputs corrupts data)
                            a1u = kp.tile([P, 2, NCH], F32, tag="a1u", name="a1u")
                            nc.vector.tensor_copy(a1u, pa)
                            r1l = kp.tile([P, 2, NCH], F32, tag="r1l", name="r1l")
                            nc.vector.tensor_scalar_mul(r1l, rps, -lam)
                        else:
                            # o' = a1*r2 - lam*a2*r1 (per-column positive
                            # rescale of o; RMSNorm cancels it). m2 first: it
                            # reads pa, which gates the next group's pv.
                            m2 = ep.tile([P, 2, NCH], F32, tag="m2", name="m2")
                            nc.vector.tensor_mul(m2, pa, r1l)
                            m1 = ep.tile([P, 2, NCH], F32, tag="m1", name="m1")
                            nc.vector.tensor_mul(m1, a1u, rps)
                            opk = kp.tile([P, 2, NCH], F32, tag="opk", name="opk")
                            nc.vector.tensor_add(opk, m1, m2)
                    # per-head RMSNorm for this 512-column chunk:
                    # rsqrt(mean o'^2) = exp(-0.5*ln(mean)); Ln+Exp cost two
                    # ACT table swaps per group (different table sets).
                    sq = ep.tile([P, 2, NCH], BF16, tag="sq", name="sq")
                    nc.vector.tensor_mul(sq, opk, opk)
                    psm = psp.tile([P, 2, NCH], F32, tag="s", name="psm")
                    for vh in range(2):
                        nc.tensor.matmul(psm[:, vh, :], ones, sq[:, vh, :],
                                         start=True, stop=True)
                    lnt = ep.tile([P, 2, NCH], F32, tag="ln", name="lnt")
                    nc.scalar.activation(lnt, psm, LOG, scale=1.0 / D2)
                    rsq = ep.tile([P, 2, NCH], F32, tag="rsq", name="rsq")
                    nc.scalar.activation(rsq, lnt, EXP, scale=-0.5)
                    on[q4] = kp.tile([P, 2, NCH], BF16, tag=f"on{q4}",
                                     name=f"on{q4}")
                    nc.vector.scalar_tensor_tensor(
                        on[q4], opk, sv, rsq, op0=MULT, op1=MULT)
                    emit_proj(q4)
                while pending:
                    pending.pop(0)()
    nc.finalize()
    return nc


def _core_inputs(x, w_qkv, w_proj, rms_scale):
    """Host-side shard prep: per-core weight slices + replicated x^T (bf16)."""
    bf = ml_dtypes.bfloat16
    xt = np.ascontiguousarray(x.reshape(T, C).T)  # [C, T]
    xtr = np.ascontiguousarray(
        xt.reshape(KSLABS, P, T // NCH, NCH).transpose(1, 2, 0, 3)
    ).astype(bf)
    sv = np.ascontiguousarray(
        (rms_scale.astype(np.float32) * np.float32(1.0 - LAMBDA_INIT)).reshape(P, 1)
    )
    maps = []
    for c in range(N_CORES):
        cols = [
            w_qkv[:, 0 * 1024 + c * P:0 * 1024 + (c + 1) * P],  # q1 heads 2c,2c+1
            w_qkv[:, 1 * 1024 + c * P:1 * 1024 + (c + 1) * P],  # q2
            w_qkv[:, 2 * 1024 + c * P:2 * 1024 + (c + 1) * P],  # k1
            w_qkv[:, 3 * 1024 + c * P:3 * 1024 + (c + 1) * P],  # k2
        ]
        wqk = np.concatenate(cols, axis=1)  # [C, 512]
        wqk = np.ascontiguousarray(
            wqk.reshape(KSLABS, P, 4 * P).transpose(1, 0, 2)).astype(bf)
        wv = w_qkv[:, 2 * C + c * 2 * D2:2 * C + (c + 1) * 2 * D2]  # [C, 256]
        wv = np.ascontiguousarray(
            wv.reshape(KSLABS, P, 2 * D2).transpose(1, 0, 2)).astype(bf)
        wp = w_proj[c * 2 * D2:(c + 1) * 2 * D2, :]  # [256, T]
        wp = np.ascontiguousarray(
            wp.reshape(2, P, T).transpose(1, 0, 2)).astype(bf)
        maps.append({"xt": xtr, "wqk": wqk, "wv": wv, "wp": wp, "sv": sv})
    return maps


def kernel(x, w_qkv, w_proj, lambda_q1, lambda_k1, lambda_q2, lambda_k2, rms_scale):
    from concourse.bass_utils import run_bass_kernel_spmd

    x = np.asarray(x, dtype=np.float32)
    w_qkv = np.asarray(w_qkv, dtype=np.float32)
    w_proj = np.asarray(w_proj, dtype=np.float32)
    rms_scale = np.asarray(rms_scale, dtype=np.float32)
    lam1 = np.exp(np.sum(np.asarray(lambda_q1) * np.asarray(lambda_k1), dtype=np.float32))
    lam2 = np.exp(np.sum(np.asarray(lambda_q2) * np.asarray(lambda_k2), dtype=np.float32))
    lam = float(lam1 - lam2 + LAMBDA_INIT)

    nc = build(lam)
    in_maps = _core_inputs(x, w_qkv, w_proj, rms_scale)
    res = run_bass_kernel_spmd(nc, in_maps, core_ids=list(range(N_CORES)))
    y = np.zeros((TT, P, T), np.float32)
    for rmap in res.results:
        y += np.asarray(rmap["y"], dtype=np.float32)
    return y.reshape(1, T, C)


# revision 15
# speedup vs baseline: 1.4726x; 1.0111x over previous
"""Trainium2 Bass kernel for DiffSelfAttention (B=1, T=2048, C=2048, 16 v-heads).

Sharding: tensor-parallel over heads across 8 NeuronCores. Core c owns
v-heads {2c, 2c+1} plus the matching q/k heads of both differential branches.
Each core computes its qkv slice, the attention for its 4 q/k heads, the
differential + per-head RMSNorm, and a partial projection
y_c = out_c @ w_proj[rows_c]. The host sums the 8 partials (unshard step).

Layout/strategy notes (v3):
  - Everything that feeds the PE is bf16 (x, w_qkv slices, q/k/v, exp(scores),
    w_proj, normed out). PSUM accumulation stays fp32, so only input rounding
    (~0.1% rms) enters; measured end-to-end rel err ~2e-3 vs the 2e-2 budget.
    bf16 also enables Fast Weight Load on LDWEIGHTS and 2x DVE modes.
  - q/k produced transposed ([d, T]); v natural ([T, d]); scores computed
    transposed ([tk, tq]) so probs@v needs no transposes anywhere.
  - Softmax divisions eliminated: RMSNorm is invariant to per-column positive
    scale, so o' = a1*r2 - lam*a2*r1 replaces a1/r1 - lam*a2/r2.
  - The attention inner loop is software-pipelined: scores(k+1) is emitted on
    the PE before pv(k), so the PE streams scores while ACT exps slab k.
  - exp-sum r is accumulated on the DVE (bf16 adds over the et ring) with one
    final ones-matmul for the cross-partition reduction; this removes 131k PE
    cycles and frees 2 PSUM banks vs accumulating r on the PE.
  - RMSNorm (exp(-0.5*ln(mean)) on ACT) and the output projection run
    per-512-column group; the projection matmuls/copies/DMAs of group g are
    emitted one step per slab-beat inside group g+1, hiding them in the
    PE/ACT slack and spreading the output DMA across the whole kernel.
  - ~36 ones@ones warm-up matmuls at kernel start keep the PE busy while the
    first weight DMAs land, so the HAM clock-gate releases (1.2->2.4 GHz)
    right as real work begins. A dummy exp prepays the ACT table load.
"""

import math

import numpy as np
import ml_dtypes

import concourse.bass as bass
import concourse.bacc as bacc
import concourse.mybir as mybir
import concourse.tile as tile

F32 = mybir.dt.float32
F32R = mybir.dt.float32r
BF16 = mybir.dt.bfloat16

T = 2048
C = 2048
N_HEAD = 16
H_DIM = 64
D2 = 2 * H_DIM  # 128 (v-head dim, also the RMS group size)
LAMBDA_INIT = 0.8 - 0.6 * math.exp(-0.3)
SCALE = 1.0 / math.sqrt(H_DIM)
P = 128
KSLABS = C // P  # 16 contraction slabs
TT = T // P  # 16 t-tiles
NCH = 512  # tq chunk = PSUM bank width in fp32
N_CORES = 8
DMAG = 2  # k-slab granularity of the startup weight/x DMA stream
NWARM = 36  # HAM warm-up matmuls

EXP = mybir.ActivationFunctionType.Exp
LOG = mybir.ActivationFunctionType.Ln
MULT = mybir.AluOpType.mult


def build(lam: float) -> bass.Bass:
    nc = bacc.Bacc("TRN2", target_bir_lowering=False, debug=False)

    xt_d = nc.dram_tensor("xt", [P, 4, KSLABS, NCH], BF16, kind="ExternalInput")
    wqk_d = nc.dram_tensor("wqk", [P, KSLABS, 4 * P], BF16, kind="ExternalInput")
    wv_d = nc.dram_tensor("wv", [P, KSLABS, 2 * D2], BF16, kind="ExternalInput")
    wp_d = nc.dram_tensor("wp", [P, 2, T], BF16, kind="ExternalInput")
    sv_d = nc.dram_tensor("sv", [P, 1], F32, kind="ExternalInput")
    y_d = nc.dram_tensor("y", [TT, P, T], BF16, kind="ExternalOutput")

    with tile.TileContext(nc) as tc:
        with tc.tile_pool(name="persist", bufs=1) as persist:
            sv = persist.tile([P, 1], F32)
            ones_f = persist.tile([P, P], F32)
            ones = persist.tile([P, P], BF16)
            actw = persist.tile([P, 1], F32)
            qk = persist.tile([P, 4, T], BF16)  # q1|q2|k1|k2, [d, T] layout
            vnat = persist.tile([P, TT, 2 * D2], BF16)  # v, [T, d] layout
            nc.sync.dma_start(out=sv, in_=sv_d[:])
            nc.vector.memset(ones_f, 1.0)
            nc.vector.tensor_copy(ones, ones_f)
            # prepay the exp table load while the PE warms up
            nc.scalar.activation(actw, ones_f[:, 0:1], EXP, scale=1.0)

            # ---------- phase 1: qkv projections ----------
            with tc.tile_pool(name="w1", bufs=1) as w1p, \
                 tc.tile_pool(name="xt", bufs=2) as xtp, \
                 tc.tile_pool(name="ps_qk", bufs=2, space="PSUM") as pqk, \
                 tc.tile_pool(name="ps_v", bufs=2, space="PSUM") as pvp:
                warm = pqk.tile([P, P], F32, tag="warm", bufs=1)
                for _ in range(NWARM):
                    nc.tensor.matmul(warm, ones, ones, start=True, stop=True)
                wqk = w1p.tile([P, KSLABS, 4 * P], BF16)
                wv = w1p.tile([P, KSLABS, 2 * D2], BF16)
                for n in range(T // NCH):  # 512-wide t chunks
                    xt = xtp.tile([P, KSLABS, NCH], BF16)
                    if n == 0:
                        # Stream wqk + x chunk 0 in small interleaved k-slab
                        # groups so the first matmul starts a few us in (one
                        # big DMA each would stall the PE ~40us at start).
                        for g in range(KSLABS // DMAG):
                            sl = slice(g * DMAG, (g + 1) * DMAG)
                            nc.sync.dma_start(out=wqk[:, sl, :], in_=wqk_d[:, sl, :])
                            nc.sync.dma_start(out=xt[:, sl, :], in_=xt_d[:, 0, sl, :])
                        nc.sync.dma_start(out=wv, in_=wv_d[:])
                    else:
                        nc.sync.dma_start(out=xt, in_=xt_d[:, n, :, :])
                    # k-major accumulation into 4 live psum banks: each
                    # arriving x/w k-slab is consumed immediately, so the
                    # chunk-0 DMA stream never stalls a psum chain.
                    pss = [pqk.tile([P, NCH], F32, tag=f"m{m}", bufs=1,
                                    name=f"psqk{m}") for m in range(4)]
                    for k in range(KSLABS):
                        for m in range(4):  # q1, q2, k1, k2
                            nc.tensor.matmul(
                                pss[m],
                                wqk[:, k, m * P:(m + 1) * P],
                                xt[:, k, :],
                                start=(k == 0),
                                stop=(k == KSLABS - 1),
                            )
                    for m in range(4):
                        nc.vector.tensor_copy(qk[:, m, n * NCH:(n + 1) * NCH], pss[m])
                    for t2 in range(NCH // P):  # t-tiles in this chunk
                        ps = pvp.tile([P, 2 * D2], F32)
                        for k in range(KSLABS):
                            nc.tensor.matmul(
                                ps,
                                xt[:, k, t2 * P:(t2 + 1) * P],
                                wv[:, k, :],
                                start=(k == 0),
                                stop=(k == KSLABS - 1),
                            )
                        nc.vector.tensor_copy(vnat[:, n * (NCH // P) + t2, :], ps)

            # ---------- phases 2+3, fused per 512-column group ----------
            with tc.tile_pool(name="wp", bufs=1) as wpp, \
                 tc.tile_pool(name="ps_s", bufs=2, space="PSUM") as psp, \
                 tc.tile_pool(name="ps_a", bufs=1, space="PSUM") as pap, \
                 tc.tile_pool(name="ps_r", bufs=1, space="PSUM") as rmsp, \
                 tc.tile_pool(name="ps_y", bufs=1, space="PSUM") as pyp, \
                 tc.tile_pool(name="exp", bufs=2) as ep, \
                 tc.tile_pool(name="keep", bufs=1) as kp, \
                 tc.tile_pool(name="ysb", bufs=3) as yp:
                wp = wpp.tile([P, 2, T], BF16)
                nc.sync.dma_start(out=wp, in_=wp_d[:])

                # Deferred projection steps: emitted one per slab-beat so the
                # proj matmuls ride in the attention loop's dependency slack
                # instead of serializing between groups.
                pending = []

                def emit_proj(q4):
                    on_t = on[q4]
                    for t2 in range(NCH // P):
                        ysb = yp.tile([P, T], BF16, tag="ysb", name=f"ysb{q4}{t2}")
                        for nchk in range(T // NCH):
                            def step(t2=t2, nchk=nchk, ysb=ysb, on_t=on_t, q4=q4):
                                py = pyp.tile([P, NCH], F32, name="py")
                                for vh in range(2):
                                    nc.tensor.matmul(
                                        py,
                                        on_t[:, vh, t2 * P:(t2 + 1) * P],
                                        wp[:, vh, nchk * NCH:(nchk + 1) * NCH],
                                        start=(vh == 0),
                                        stop=(vh == 1),
                                    )
                                nc.vector.tensor_copy(
                                    ysb[:, nchk * NCH:(nchk + 1) * NCH], py)
                                if nchk == T // NCH - 1:
                                    nc.sync.dma_start(
                                        out=y_d[q4 * (NCH // P) + t2], in_=ysb)
                            pending.append(step)

                on = {}
                for q4 in range(4):
                    c0 = q4 * NCH
                    a1u = None
                    r1l = None
                    opk = None
                    for br in range(2):
                        pa = pap.tile([P, 2, NCH], F32, tag="pa", name="pa")
                        racc = kp.tile([P, 2, NCH], BF16, tag=f"racc{br}",
                                       name=f"racc{br}")

                        def scores(k):
                            ps = psp.tile([P, 2, NCH], F32, tag="s", name="ps")
                            for vh in range(2):
                                rows = slice(vh * H_DIM, (vh + 1) * H_DIM)
                                nc.tensor.matmul(
                                    ps[:, vh, :],
                                    qk[rows, 2 + br, k * P:(k + 1) * P],
                                    qk[rows, br, c0:c0 + NCH],
                                    start=True,
                                    stop=True,
                                )
                            return ps

                        ps_cur = scores(0)
                        for k in range(TT):  # tk slabs
                            ps_nxt = scores(k + 1) if k + 1 < TT else None
                            et = ep.tile([P, 2, NCH], BF16, tag="er", bufs=12,
                                         name="et")
                            nc.scalar.activation(et, ps_cur, EXP, scale=SCALE)
                            for vh in range(2):
                                nc.tensor.matmul(
                                    pa[:, vh, :],
                                    vnat[:, k, vh * D2:(vh + 1) * D2],
                                    et[:, vh, :],
                                    start=(k == 0),
                                    stop=(k == TT - 1),
                                )
                            if k == 0:
                                nc.vector.tensor_copy(racc, et)
                            else:
                                nc.vector.tensor_add(racc, racc, et)
                            if k % 2 == 1 and pending:
                                # one deferred proj step every other beat: the
                                # DVE can't absorb a copy every beat on top of
                                # the racc chain
                                pending.pop(0)()
                            ps_cur = ps_nxt
                        # cross-partition reduce of the exp-sums (broadcasts
                        # the column sums to all 128 partitions). Lives in the
                        # dedicated 1-bank rms pool so the scores ring is
                        # never blocked across group boundaries.
                        if br == 0:
                            # keep unnormalized a1 and -lam*r1 for branch 2
                            # (a1u stays f32: tensor_mul encodes one source
                            # dtype, so mixing bf16/f32 inputs corrupts data)
                            a1u = kp.tile([P, 2, NCH], F32, tag="a1u", name="a1u")
                            nc.vector.tensor_copy(a1u, pa)
                            r1l = kp.tile([P, 2, NCH], F32, tag="r1l", name="r1l")
                            for vh in range(2):
                                rpv = rmsp.tile([P, NCH], F32, tag="r", name="rpv")
                                nc.tensor.matmul(rpv, ones, racc[:, vh, :],
                                                 start=True, stop=True)
                                nc.vector.tensor_scalar_mul(r1l[:, vh, :], rpv, -lam)
                        else:
                            # o' = a1*r2 - lam*a2*r1 (per-column positive
                            # rescale of o; RMSNorm cancels it). m2 first: it
                            # reads pa, which gates the next group's pv.
                            m2 = ep.tile([P, 2, NCH], F32, tag="m2", name="m2")
                            nc.vector.tensor_mul(m2, pa, r1l)
                            m1 = ep.tile([P, 2, NCH], F32, tag="m1", name="m1")
                            for vh in range(2):
                                rpv = rmsp.tile([P, NCH], F32, tag="r", name="rpv")
                                nc.tensor.matmul(rpv, ones, racc[:, vh, :],
                                                 start=True, stop=True)
                                nc.vector.tensor_mul(m1[:, vh, :], a1u[:, vh, :], rpv)
                            opk = kp.tile([P, 2, NCH], F32, tag="opk", name="opk")
                            nc.vector.tensor_add(opk, m1, m2)
                    # per-head RMSNorm for this 512-column chunk:
                    # rsqrt(mean o'^2) = exp(-0.5*ln(mean)); Ln+Exp cost two
                    # ACT table swaps per group (different table sets).
                    sq = ep.tile([P, 2, NCH], BF16, tag="sq", name="sq")
                    nc.vector.tensor_mul(sq, opk, opk)
                    lnt = ep.tile([P, 2, NCH], F32, tag="ln", name="lnt")
                    for vh in range(2):
                        psm = rmsp.tile([P, NCH], F32, tag="r", name="psm")
                        nc.tensor.matmul(psm, ones, sq[:, vh, :],
                                         start=True, stop=True)
                        nc.scalar.activation(lnt[:, vh, :], psm, LOG, scale=1.0 / D2)
                    rsq = ep.tile([P, 2, NCH], F32, tag="rsq", name="rsq")
                    nc.scalar.activation(rsq, lnt, EXP, scale=-0.5)
                    on[q4] = kp.tile([P, 2, NCH], BF16, tag=f"on{q4}",
                                     name=f"on{q4}")
                    nc.vector.scalar_tensor_tensor(
                        on[q4], opk, sv, rsq, op0=MULT, op1=MULT)
                    emit_proj(q4)
                while pending:
                    pending.pop(0)()
    nc.finalize()
    return nc


def _core_inputs(x, w_qkv, w_proj, rms_scale):
    """Host-side shard prep: per-core weight slices + replicated x^T (bf16)."""
    bf = ml_dtypes.bfloat16
    xt = np.ascontiguousarray(x.reshape(T, C).T)  # [C, T]
    xtr = np.ascontiguousarray(
        xt.reshape(KSLABS, P, T // NCH, NCH).transpose(1, 2, 0, 3)
    ).astype(bf)
    sv = np.ascontiguousarray(
        (rms_scale.astype(np.float32) * np.float32(1.0 - LAMBDA_INIT)).reshape(P, 1)
    )
    maps = []
    for c in range(N_CORES):
        cols = [
            w_qkv[:, 0 * 1024 + c * P:0 * 1024 + (c + 1) * P],  # q1 heads 2c,2c+1
            w_qkv[:, 1 * 1024 + c * P:1 * 1024 + (c + 1) * P],  # q2
            w_qkv[:, 2 * 1024 + c * P:2 * 1024 + (c + 1) * P],  # k1
            w_qkv[:, 3 * 1024 + c * P:3 * 1024 + (c + 1) * P],  # k2
        ]
        wqk = np.concatenate(cols, axis=1)  # [C, 512]
        wqk = np.ascontiguousarray(
            wqk.reshape(KSLABS, P, 4 * P).transpose(1, 0, 2)).astype(bf)
        wv = w_qkv[:, 2 * C + c * 2 * D2:2 * C + (c + 1) * 2 * D2]  # [C, 256]
        wv = np.ascontiguousarray(
            wv.reshape(KSLABS, P, 2 * D2).transpose(1, 0, 2)).astype(bf)
        wp = w_proj[c * 2 * D2:(c + 1) * 2 * D2, :]  # [256, T]
        wp = np.ascontiguousarray(
            wp.reshape(2, P, T).transpose(1, 0, 2)).astype(bf)
        maps.append({"xt": xtr, "wqk": wqk, "wv": wv, "wp": wp, "sv": sv})
    return maps


def kernel(x, w_qkv, w_proj, lambda_q1, lambda_k1, lambda_q2, lambda_k2, rms_scale):
    from concourse.bass_utils import run_bass_kernel_spmd

    x = np.asarray(x, dtype=np.float32)
    w_qkv = np.asarray(w_qkv, dtype=np.float32)
    w_proj = np.asarray(w_proj, dtype=np.float32)
    rms_scale = np.asarray(rms_scale, dtype=np.float32)
    lam1 = np.exp(np.sum(np.asarray(lambda_q1) * np.asarray(lambda_k1), dtype=np.float32))
    lam2 = np.exp(np.sum(np.asarray(lambda_q2) * np.asarray(lambda_k2), dtype=np.float32))
    lam = float(lam1 - lam2 + LAMBDA_INIT)

    nc = build(lam)
    in_maps = _core_inputs(x, w_qkv, w_proj, rms_scale)
    res = run_bass_kernel_spmd(nc, in_maps, core_ids=list(range(N_CORES)))
    y = np.zeros((TT, P, T), np.float32)
    for rmap in res.results:
        y += np.asarray(rmap["y"], dtype=np.float32)
    return y.reshape(1, T, C)
